# revision 1
# baseline (speedup 1.0000x reference)
"""Trainium2 Bass kernel for nn_BodyKinematics (batched tree forward kinematics).

Contract: kernel(**inputs) takes the FULL unsharded inputs as numpy arrays and
returns the FULL output (B, N, 4, 4) float32.  Internally the batch dim is
sharded across 8 NeuronCores (pure data parallelism); the tiny per-edge
parameters are replicated.

Math (matches the jax reference):
  theta = tanh(log_angles) * scale + offset            # (B, 3E)
  per edge e: r = Rx(th_x) @ Ry(th_y) @ Rz(th_z)       # axes are e_x, e_y, e_z
  local_e  = [r | 0; 0 1] @ tip_to_base[e]             # affine 3x4 is enough
  tree: W_0 = I, W_n = W_parent(n) @ local_{n-1}       # parent(n) = (n-1)//2
  output: W as 4x4 with constant bottom row (0,0,0,1)

Device layout (per core, 512 batch rows = 4 subtiles x 128 partitions):
  partitions = batch-within-subtile, free dim = per-edge structure.
  locals tile (per sub):  [128, E*12]   (e, i, l) row-major
  worlds tile (per sub):  [128, N*16]   (n, i, l) -> 16KB contiguous per batch
                                         row for efficient output DMA
"""

import os
import sys

for _p in ("/opt/trn_rl_repo",):
    if _p not in sys.path and os.path.isdir(_p):
        sys.path.insert(0, _p)

import numpy as np

B, E, N = 4096, 255, 256
J = 3 * E           # 765 angles
NCORE, P, NSUB = 8, 128, 4
BPC = P * NSUB      # 512 batch rows per core
PI = float(np.pi)

# engine assignment: "v" = VectorE (DVE), "g" = GpSimd (POOL).
# BC_ENG: per op-triple (tA_mul, indep_mul, combine); same for all subs.
BC_ENG = [("v", "v", "v")] * 6
# tree per-level op classes: k=0 muls L/R, k=1 muls L/R + add, k=2 muls L/R
# + add, translation add.
TREE_ENG = {"m0L": "v", "m0R": "v", "t1L": "v", "t1R": "v", "a1": "v",
            "t2L": "v", "t2R": "v", "a2": "v", "tr": "v"}
MEMSET_ENG = "v"
TREE_SUB_ENG = ["g", "v", "v", "v"]
# per-sub, per-level engine override (7 levels + tail); None -> use
# TREE_SUB_ENG for every level
TREE_LVL_ENG = {}
BC_ALT = False    # alternate engine roles between BC groups
TREE_ALT = False  # alternate engine roles between tree groups
GS = 1      # subs per BC op-group
TGS = 2     # subs per tree op-group
COS_MODE = "abs"
STAGE_LP = False  # ACT-stage locals into PSUM for DVE tree levels          # "abs" (1 TS op @2x) or "wrap" (custom DVE op @1x)
# debug: skip stages to attribute time in TimelineSim
SKIP = set()              # subset of {"A", "BC", "TREE", "OUT"}

_state: dict = {}


# --------------------------------------------------------------------------- #
# numpy fallback (exact float32 port of the reference) — used only if the
# inputs don't match the structure the device kernel was built for.
# --------------------------------------------------------------------------- #
def _np_skew(a):
    x, y, z = a[..., 0], a[..., 1], a[..., 2]
    zero = np.zeros_like(x)
    return np.stack([
        np.stack([zero, -z, y], -1),
        np.stack([z, zero, -x], -1),
        np.stack([-y, x, zero], -1)], -2)


def _np_fallback(log_angles, tip_to_base, rot_axes, rot_constraints):
    la = log_angles.astype(np.float32)
    b, e3 = la.shape
    e = e3 // 3
    n = e + 1
    theta = np.tanh(la) * rot_constraints[:, 0] + rot_constraints[:, 1]
    K = _np_skew(rot_axes.astype(np.float32))
    K2 = np.einsum('mij,mjk->mik', K, K).astype(np.float32)
    s = np.sin(theta)[..., None, None]
    c = (1.0 - np.cos(theta))[..., None, None]
    I3 = np.eye(3, dtype=np.float32)
    rots = (I3 + s * K + c * K2).reshape(b, e, 3, 3, 3).astype(np.float32)
    r = np.einsum('beij,bejk,bekl->beil', rots[:, :, 0], rots[:, :, 1],
                  rots[:, :, 2]).astype(np.float32)
    T = np.zeros((b, e, 4, 4), np.float32)
    T[..., :3, :3] = r
    T[..., 3, 3] = 1.0
    local = np.einsum('beij,ejk->beik', T,
                      tip_to_base.astype(np.float32)).astype(np.float32)
    worlds = np.zeros((b, n, 4, 4), np.float32)
    worlds[:, 0] = np.eye(4, dtype=np.float32)
    for i in range(1, n):
        par = (i - 1) // 2
        worlds[:, i] = (worlds[:, par] @ local[:, i - 1]).astype(np.float32)
    return worlds


# --------------------------------------------------------------------------- #
# device kernel build
# --------------------------------------------------------------------------- #
def _build_nc(general_constraints: bool, sc_const: float, of_const: float,
              loop_n: int = 1):
    import concourse.bacc as bacc
    import concourse.mybir as mybir
    from concourse.tile import TileContext
    import concourse.bass as bass
    from contextlib import ExitStack

    f32 = mybir.dt.float32
    i32 = mybir.dt.int32
    Alu = mybir.AluOpType
    AFT = mybir.ActivationFunctionType

    nc = bacc.Bacc("TRN2", target_bir_lowering=False, debug=False)

    la_d = nc.dram_tensor("la", [BPC, J], f32, kind="ExternalInput")
    tip_d = nc.dram_tensor("tip", [1, E * 12], f32, kind="ExternalInput")
    if general_constraints:
        cs_d = nc.dram_tensor("cs", [1, J], f32, kind="ExternalInput")
        co_d = nc.dram_tensor("co", [1, J], f32, kind="ExternalInput")
    out_d = nc.dram_tensor("out", [BPC, N * 16], f32, kind="ExternalOutput")

    def eng(tag):
        return nc.vector if tag == "v" else nc.gpsimd

    with TileContext(nc) as tc:
        with tc.tile_pool(name="main", bufs=1) as pool, \
             tc.tile_pool(name="scr", bufs=2) as scr, \
             ExitStack() as _loop_ctx:
            if loop_n > 1:
                _loop_ctx.enter_context(tc.For_i(0, loop_n, 1))

            la_t = pool.tile([P, NSUB * J], f32)    # input; reused as |t|
            th_t = pool.tile([P, NSUB * J], f32)    # tanh
            tip_t = pool.tile([P, E * 12], f32)     # broadcast tip rows
            loc_t = [pool.tile([P, E * 12], f32, tag=f"loc{s}",
                               name=f"loc{s}") for s in range(NSUB)]
            w_t = [pool.tile([P, N * 16], f32, tag=f"w{s}", name=f"w{s}")
                   for s in range(NSUB)]
            if general_constraints:
                cs_t = pool.tile([P, J], f32)
                co_t = pool.tile([P, J], f32)

            # ---------------- input DMAs ----------------
            la_v = la_d[:].rearrange("(s p) j -> p s j", p=P)    # [128, 4, 765]
            for s in range(NSUB):
                nc.sync.dma_start(la_t[:, s * J:(s + 1) * J], la_v[:, s])
            tq = (E * 12) // 4
            for c in range(4):
                tip_src = bass.AP(tip_d, c * tq, [[0, P], [1, tq]])
                nc.sync.dma_start(tip_t[:, c * tq:(c + 1) * tq], tip_src)

            if general_constraints:
                nc.sync.dma_start(cs_t[:], bass.AP(cs_d, 0, [[0, P], [1, J]]))
                nc.sync.dma_start(co_t[:], bass.AP(co_d, 0, [[0, P], [1, J]]))

            # ---------------- stage A: theta -> tanh ----------------
            act = nc.scalar.activation
            for s in range(NSUB):
                act(th_t[:, s * J:(s + 1) * J], la_t[:, s * J:(s + 1) * J],
                    AFT.Tanh)
            if general_constraints:
                for s in range(NSUB):
                    blk = th_t[:, s * J:(s + 1) * J]
                    nc.vector.tensor_tensor(blk, blk, cs_t[:], Alu.mult)
                    nc.vector.tensor_tensor(blk, blk, co_t[:], Alu.add)
                scv, ofv = 1.0, 0.0
            else:
                scv, ofv = sc_const, of_const
            if ofv == 0.0:
                ofv_ap = None
            else:
                ofv_t = pool.tile([P, 1], f32)
                nc.gpsimd.memset(ofv_t[:], ofv)
                ofv_ap = ofv_t[:]
            use_abs = (COS_MODE == "abs" and ofv == 0.0)
            if use_abs:
                hpi_t = pool.tile([P, 1], f32)
                nc.gpsimd.memset(hpi_t[:], PI / 2.0)
                # |t| for the whole tile in one 2x op (cos input)
                nc.vector.tensor_scalar(
                    la_t[:].bitcast(i32), th_t[:].bitcast(i32),
                    0x7FFFFFFF, None, Alu.bitwise_and)
            else:
                nc.vector.add_range_wrap(la_t[:], th_t[:],
                                         (ofv + PI / 2.0) / scv, PI / scv,
                                         2.0 * PI / scv)

            # ---------------- BC + tree, pipelined per sub ----------------
            tip3 = tip_t[:].rearrange("p (e i l) -> p e i l", e=E, i=3, l=4)
            T0, T1, T2 = (tip3[:, :, i, :] for i in range(3))
            _bc_ps = tc.tile_pool(name="bcps", bufs=2, space="PSUM")
            psp2 = _bc_ps.__enter__()
            _bc_ps1 = tc.tile_pool(name="bcps1", bufs=1, space="PSUM")
            psp = _bc_ps1.__enter__()
            tA = psp.tile([P, E * 4], f32)           # shared PSUM scratch
            tAv = tA[:].rearrange("p (e l) -> p e l", e=E, l=4)

            for s in range(NSUB):
                j0 = s * J
                # per-sub sin/cos in PSUM, written by ACT (own ports)
                sc_ps = psp2.tile([P, 2 * J], f32, tag="scps",
                                  name=f"scps_{s}")
                sin_ap = sc_ps[:, 0:J]
                cos_ap = sc_ps[:, J:2 * J]
                if ofv_ap is None:
                    act(sin_ap, th_t[:, j0:j0 + J], AFT.Sin, scale=scv)
                else:
                    act(sin_ap, th_t[:, j0:j0 + J], AFT.Sin, bias=ofv_ap,
                        scale=scv)
                if use_abs:
                    act(cos_ap, la_t[:, j0:j0 + J], AFT.Sin, bias=hpi_t[:],
                        scale=-scv)
                else:
                    act(cos_ap, la_t[:, j0:j0 + J], AFT.Sin, scale=scv)

                def trig(base, axis):
                    return base[:, axis::3].to_broadcast([P, E, 4])

                sx, sy, sz = (trig(sin_ap, a) for a in range(3))
                cx, cy, cz = (trig(cos_ap, a) for a in range(3))

                r0 = scr.tile([P, E * 4], f32, tag="r0", name=f"r0_{s}")
                r1 = scr.tile([P, E * 4], f32, tag="r1", name=f"r1_{s}")
                q2 = scr.tile([P, E * 4], f32, tag="q2", name=f"q2_{s}")
                r0v, r1v, q2v = (
                    t[:].rearrange("p (e l) -> p e l", e=E, l=4)
                    for t in (r0, r1, q2))
                loc4 = loc_t[s][:].rearrange("p (e i l) -> p e i l",
                                             e=E, i=3, l=4)
                L0, L1, L2 = (loc4[:, :, i, :] for i in range(3))

                # all on DVE; every op reads <=1 SBUF stream (trig + tA are
                # PSUM), so POOL can run other subs' trees concurrently.
                tt = nc.vector.tensor_tensor
                triples = [
                    (cz, T0, sz, T1, r0v, Alu.subtract, True),
                    (sz, T0, cz, T1, r1v, Alu.add, False),
                    (cy, r0v, sy, T2, L0, Alu.add, False),
                    (sy, r0v, cy, T2, q2v, Alu.subtract, False),
                    (cx, r1v, sx, q2v, L1, Alu.subtract, True),
                    (sx, r1v, cx, q2v, L2, Alu.add, False),
                ]
                for (a, b, c, d, dst, op, ta_first) in triples:
                    tt(tAv, a, b, Alu.mult)
                    tt(dst, c, d, Alu.mult)
                    if ta_first:
                        tt(dst, tAv, dst, op)
                    else:
                        tt(dst, dst, tAv, op)

            _bc_ps1.__exit__(None, None, None)
            _bc_ps.__exit__(None, None, None)
            _tr_ps = tc.tile_pool(name="trps", bufs=2, space="PSUM")
            pst = _tr_ps.__enter__()

            # ---------------- tree, per sub; engine per TREE_SUB_ENG -------
            for s in range(NSUB):
                etag = TREE_SUB_ENG[s]
                lvl_tags = TREE_LVL_ENG.get(
                    s, [etag] * 8)
                ev = eng(etag)
                wt = w_t[s]
                lt = loc_t[s]
                w4 = wt[:].rearrange("p (n i l) -> p n i l", n=N, i=4, l=4)
                loc4 = lt[:].rearrange("p (e i l) -> p e i l", e=E, i=3, l=4)
                wap = wt[:]
                lap = lt[:]
                wpdim = list(wap.ap[0])
                lpdim = list(lap.ap[0])
                woff = wap.offset
                loff = lap.offset

                def wAP(off, dims):
                    return bass.AP(wap.tensor, woff + off,
                                   [list(wpdim)] + dims)

                def lAP(off, dims):
                    return bass.AP(lap.tensor, loff + off,
                                   [list(lpdim)] + dims)

                ev.memset(w4[:, :, 3, 0:3], 0.0)
                ev.memset(w4[:, :, 3, 3], 1.0)
                ev.memset(w4[:, 0, 0:3, :], 0.0)
                ev.memset(wAP(0, [[5, 3]]), 1.0)      # root rot diag
                ev.tensor_copy(w4[:, 1:3, 0:3, :], loc4[:, 0:2, :, :])

                tmps = {}
                for tg in set(lvl_tags):
                    if tg == "v" and STAGE_LP:
                        tmps["v"] = pst.tile([P, 64 * 12], f32,
                                             tag="ttmp_ps",
                                             name=f"ttmpv_{s}")
                    else:
                        tmps[tg] = scr.tile([P, 64 * 12], f32, tag="ttmp",
                                            name=f"ttmpg_{s}")

                for li, (lo, hi) in enumerate(
                        [(3, 7), (7, 15), (15, 31), (31, 63),
                         (63, 127), (127, 191), (191, 255)]):
                    ltag = lvl_tags[li]
                    tt = eng(ltag).tensor_tensor
                    tmp = tmps[ltag]
                    use_psum = (ltag == "v") and STAGE_LP
                    m = hi - lo
                    q = m // 2
                    plo = (lo - 1) // 2
                    if use_psum:
                        # stage this level's locals into PSUM via ACT so the
                        # DVE muls read only one SBUF stream
                        lp = pst.tile([P, 64 * 12], f32, tag="lp_ps",
                                      name=f"lp_{s}_{lo}")
                        nc.scalar.copy(lp[:, 0:m * 12],
                                       lt[:, (lo - 1) * 12:(hi - 1) * 12])
                        lsrc_base = lp[:]
                        lsoff = lp[:].offset
                        lspd = list(lp[:].ap[0])

                        def lsAP(off, dims):
                            return bass.AP(lsrc_base.tensor, lsoff + off,
                                           [list(lspd)] + dims)
                    tmpv = tmp[:].rearrange("p (n i l) -> p n i l",
                                            n=64, i=3, l=4)[:, 0:m, :, :]
                    for k in range(3):
                        wp = w4[:, plo:plo + q, 0:3, k].to_broadcast(
                            [P, q, 3, 4])
                        for side in (0, 1):
                            if use_psum:
                                lsrc = lsAP(side * 12 + k * 4,
                                            [[24, q], [0, 3], [1, 4]])
                            else:
                                lsrc = lAP((lo - 1 + side) * 12 + k * 4,
                                           [[24, q], [0, 3], [1, 4]])
                            if k == 0:
                                dst = wAP((lo + side) * 16,
                                          [[32, q], [4, 3], [1, 4]])
                            else:
                                tap = tmp[:]
                                dst = bass.AP(tap.tensor,
                                              tap.offset + side * 12,
                                              [list(tap.ap[0]),
                                               [24, q], [4, 3], [1, 4]])
                            tt(dst, wp, lsrc, Alu.mult)
                        if k > 0:
                            wdst = w4[:, lo:hi, 0:3, :]
                            tt(wdst, wdst, tmpv, Alu.add)
                    wtr = wAP(lo * 16 + 3, [[32, q], [16, 2], [4, 3]])
                    ptr = wAP(plo * 16 + 3, [[16, q], [0, 2], [4, 3]])
                    tt(wtr, wtr, ptr, Alu.add)

                # node 255 (single left child of 127)
                tt = eng(lvl_tags[7]).tensor_tensor
                tmp = tmps[lvl_tags[7]]
                for k in range(3):
                    wpk = w4[:, 127, 0:3, k].to_broadcast([P, 3, 4])
                    lsrc = lAP(254 * 12 + k * 4, [[0, 3], [1, 4]])
                    if k == 0:
                        tt(w4[:, 255, 0:3, :], wpk, lsrc, Alu.mult)
                    else:
                        t255 = tmp[:].rearrange("p (n i l) -> p n i l",
                                                n=64, i=3, l=4)[:, 0, :, :]
                        tt(t255, wpk, lsrc, Alu.mult)
                        tt(w4[:, 255, 0:3, :], w4[:, 255, 0:3, :], t255,
                           Alu.add)
                tt(wAP(255 * 16 + 3, [[4, 3]]),
                   wAP(255 * 16 + 3, [[4, 3]]),
                   wAP(127 * 16 + 3, [[4, 3]]), Alu.add)

            _tr_ps.__exit__(None, None, None)

            # ---------------- output DMAs ----------------
            out_v = out_d[:].rearrange("(s p) m -> p s m", p=P)  # [128,4,4096]
            for s in range(NSUB):
                for h in range(2):
                    for c in range(4):
                        a0 = h * 2048 + c * 512
                        nc.sync.dma_start(out_v[:, s, a0:a0 + 512],
                                          w_t[s][:, a0:a0 + 512])

    nc.compile()
    return nc


# --------------------------------------------------------------------------- #
# cached PJRT runner (axon path) — compile once, execute per call
# --------------------------------------------------------------------------- #
def _get_runner(general_constraints, sc_const, of_const, loop_n=1):
    key = ("runner", general_constraints, round(sc_const, 6), round(of_const, 6), loop_n)
    if key in _state:
        return _state[key]

    import jax
    from jax.sharding import Mesh, PartitionSpec, NamedSharding
    from jax.experimental.shard_map import shard_map
    import concourse.mybir as mybir
    from concourse import bass2jax

    nc = _build_nc(general_constraints, sc_const, of_const, loop_n)
    bass2jax.install_neuronx_cc_hook()

    part_name = (nc.partition_id_tensor.name
                 if nc.partition_id_tensor is not None else None)
    in_names, out_names, out_avals = [], [], []
    for alloc in nc.m.functions[0].allocations:
        if not isinstance(alloc, mybir.MemoryLocationSet):
            continue
        name = alloc.memorylocations[0].name
        if alloc.kind == "ExternalInput":
            if name != part_name:
                in_names.append(name)
        elif alloc.kind == "ExternalOutput":
            out_names.append(name)
            out_avals.append(jax.core.ShapedArray(
                tuple(alloc.tensor_shape), mybir.dt.np(alloc.dtype)))
    n_params = len(in_names)
    all_in = in_names + out_names
    if part_name is not None:
        all_in = all_in + [part_name]

    def _body(*args):
        operands = list(args)
        if part_name is not None:
            operands.append(bass2jax.partition_id_tensor())
        outs = bass2jax._bass_exec_p.bind(
            *operands,
            out_avals=tuple(out_avals),
            in_names=tuple(all_in),
            out_names=tuple(out_names),
            lowering_input_output_aliases=(),
            sim_require_finite=True,
            sim_require_nnan=True,
            nc=nc,
        )
        return tuple(outs)

    devices = jax.devices()[:NCORE]
    mesh = Mesh(np.asarray(devices), ("core",))
    nin = n_params + len(out_names)
    sharded = jax.jit(
        shard_map(_body, mesh=mesh,
                  in_specs=(PartitionSpec("core"),) * nin,
                  out_specs=(PartitionSpec("core"),) * len(out_names),
                  check_rep=False),
        donate_argnums=tuple(range(n_params, nin)),
        keep_unused=True,
    )
    shard0 = NamedSharding(mesh, PartitionSpec("core"))

    def _make_zeros():
        return jax.jit(
            lambda: jax.numpy.zeros((NCORE * BPC, N * 16), np.float32),
            out_shardings=shard0)()

    runner = (sharded, in_names, _make_zeros)
    _state[key] = runner
    return runner


def _run_device(log_angles, tip_rows, cs, co, general_constraints,
                sc_const, of_const, loop_n=1):
    sharded, in_names, make_zeros = _get_runner(
        general_constraints, sc_const, of_const, loop_n)
    feed = {
        "la": np.ascontiguousarray(log_angles, dtype=np.float32),
        "tip": np.broadcast_to(tip_rows.reshape(1, E * 12),
                               (NCORE, E * 12)).copy(),
    }
    if general_constraints:
        feed["cs"] = np.broadcast_to(cs.reshape(1, J), (NCORE, J)).copy()
        feed["co"] = np.broadcast_to(co.reshape(1, J), (NCORE, J)).copy()
    args = [feed[name] for name in in_names]
    out = sharded(*args, make_zeros())[0]
    return np.asarray(out).reshape(B, N, 4, 4)


def _bench_device(log_angles, tip_rows, sc_const, of_const, loop_n, reps):
    """Device-only timing: inputs stay on device, outputs never fetched."""
    import time
    import jax

    sharded, in_names, make_zeros = _get_runner(False, sc_const, of_const,
                                                loop_n)
    feed = {
        "la": np.ascontiguousarray(log_angles, dtype=np.float32),
        "tip": np.broadcast_to(tip_rows.reshape(1, E * 12),
                               (NCORE, E * 12)).copy(),
    }
    args = [jax.device_put(feed[n]) for n in in_names]
    # warmup (compile + first exec)
    jax.block_until_ready(sharded(*args, make_zeros()))
    ts = []
    for _ in range(reps):
        z = make_zeros()
        jax.block_until_ready(z)
        t0 = time.time()
        jax.block_until_ready(sharded(*args, z))
        ts.append(time.time() - t0)
    return min(ts)


# --------------------------------------------------------------------------- #
# public entry point
# --------------------------------------------------------------------------- #
def kernel(log_angles, tip_to_base, rot_axes, rot_constraints):
    log_angles = np.asarray(log_angles)
    tip_to_base = np.asarray(tip_to_base)
    rot_axes = np.asarray(rot_axes)
    rot_constraints = np.asarray(rot_constraints)

    expected_shapes = (log_angles.shape == (B, J)
                       and tip_to_base.shape == (E, 4, 4)
                       and rot_axes.shape == (J, 3)
                       and rot_constraints.shape == (J, 2))
    eye_tiled = np.tile(np.eye(3, dtype=np.float32), (E, 1)) \
        if expected_shapes else None
    euler = expected_shapes and np.allclose(rot_axes, eye_tiled, atol=1e-6)
    if not euler:
        return _np_fallback(log_angles, tip_to_base, rot_axes, rot_constraints)

    sc = rot_constraints[:, 0].astype(np.float32)
    of = rot_constraints[:, 1].astype(np.float32)
    const_ok = (np.all(sc == sc[0]) and np.all(of == of[0])
                and float(sc[0]) > 1e-3
                and abs(float(sc[0])) + abs(float(of[0])) <= PI + 1e-4)
    if not const_ok:
        # untested-on-device parameter regime: use the exact host fallback
        return _np_fallback(log_angles, tip_to_base, rot_axes,
                            rot_constraints)

    tip_rows = np.ascontiguousarray(
        tip_to_base[:, :3, :], dtype=np.float32)          # (E, 3, 4)

    out = _run_device(log_angles, tip_rows, None, None, False,
                      float(sc[0]), float(of[0]))
    return out



# revision 9
# speedup vs baseline: 1.2876x; 1.2876x over previous
"""Trainium2 Bass kernel for nn_BodyKinematics (batched tree forward kinematics).

Contract: kernel(**inputs) takes the FULL unsharded inputs as numpy arrays and
returns the FULL output (B, N, 4, 4) float32.  Batch is sharded across 8
NeuronCores (pure data parallelism); per-edge parameters replicated.

Math (matches the jax reference):
  theta = tanh(log_angles) * scale                     # offset == 0 fast path
  per edge e: r = Rx @ Ry @ Rz ; local = r @ tip       # affine 3x4
  tree: W_n = W_parent(n) @ local_{n-1}, parent(n) = (n-1)//2

Device layout (fp16; per core 512 batch rows = NSUB subtiles x 128 partitions
x S_B "lanes"; partition = batch-within-subtile, lanes = extra batch rows
interleaved into the free dim so tree ops fuse across them):
  e' = S_B*e + lane   (interleaved edge index, M = S_B*E per subtile)
  theta/trig tiles:  [128, 3*M]       axis-major, e' innermost (packed)
  locT tile:         [128, 12*M]      k*4M + l*M + e'   (BC output, packed)
  lR tile:           [128, 36*M]      e'*36 + k*12 + l*3 + i  (locals
                      replicated x3 over i so tree muls have packed last dim)
  w tile:            [128, 255*12*S_B] node n>=1 at (n-1)*12S_B + lane*12
                      + l*3 + i      (TRANSPOSED 3x4: (l,i), no bottom row)
All heavy ops are fp16 with packed last dims -> DVE 2x_1p mode.
Output DMA'd as fp16 in device layout; host unpacks to (B,N,4,4) fp32.
"""

import os
import sys

for _p in ("/opt/trn_rl_repo",):
    if _p not in sys.path and os.path.isdir(_p):
        sys.path.insert(0, _p)

import numpy as np

B, E, N = 4096, 255, 256
J = 3 * E
NCORE, P = 8, 128
S_B = 2                 # batch lanes interleaved per subtile
NSUB = 4 // S_B         # subtiles per core
M = S_B * E             # interleaved edges per subtile
BPC = P * S_B * NSUB    # 512 batch rows per core
OUTC = NSUB * 255 * 12 * S_B   # out cols per partition row (fp16)
PI = float(np.pi)

# ---- engine split knobs ----
BC_POOL_FRAC = 0.22     # e'-range tail of every BC op -> POOL
TREE_POOL_FRAC = 0.20   # node-range tail of big tree levels -> POOL
TREE_POOL_MIN_M = 16    # only split levels with at least this many nodes
REP_SPLIT = (1.0, 0.0, 0.0)   # lR replication: ACT, POOL, DVE shares

_state: dict = {}


# --------------------------------------------------------------------------- #
# numpy fallback (exact float32 port of the reference)
# --------------------------------------------------------------------------- #
def _np_skew(a):
    x, y, z = a[..., 0], a[..., 1], a[..., 2]
    zero = np.zeros_like(x)
    return np.stack([
        np.stack([zero, -z, y], -1),
        np.stack([z, zero, -x], -1),
        np.stack([-y, x, zero], -1)], -2)


def _np_fallback(log_angles, tip_to_base, rot_axes, rot_constraints):
    la = log_angles.astype(np.float32)
    b, e3 = la.shape
    e = e3 // 3
    n = e + 1
    theta = np.tanh(la) * rot_constraints[:, 0] + rot_constraints[:, 1]
    K = _np_skew(rot_axes.astype(np.float32))
    K2 = np.einsum('mij,mjk->mik', K, K).astype(np.float32)
    s = np.sin(theta)[..., None, None]
    c = (1.0 - np.cos(theta))[..., None, None]
    I3 = np.eye(3, dtype=np.float32)
    rots = (I3 + s * K + c * K2).reshape(b, e, 3, 3, 3).astype(np.float32)
    r = np.einsum('beij,bejk,bekl->beil', rots[:, :, 0], rots[:, :, 1],
                  rots[:, :, 2]).astype(np.float32)
    T = np.zeros((b, e, 4, 4), np.float32)
    T[..., :3, :3] = r
    T[..., 3, 3] = 1.0
    local = np.einsum('beij,ejk->beik', T,
                      tip_to_base.astype(np.float32)).astype(np.float32)
    worlds = np.zeros((b, n, 4, 4), np.float32)
    worlds[:, 0] = np.eye(4, dtype=np.float32)
    for i in range(1, n):
        par = (i - 1) // 2
        worlds[:, i] = (worlds[:, par] @ local[:, i - 1]).astype(np.float32)
    return worlds


# --------------------------------------------------------------------------- #
# device kernel build
# --------------------------------------------------------------------------- #
def _build_nc(sc_const: float, loop_n: int = 1):
    import concourse.bacc as bacc
    import concourse.mybir as mybir
    from concourse.tile import TileContext
    import concourse.bass as bass
    from contextlib import ExitStack

    f32 = mybir.dt.float32
    f16 = mybir.dt.float16
    i16 = mybir.dt.int16
    Alu = mybir.AluOpType
    AFT = mybir.ActivationFunctionType

    nc = bacc.Bacc("TRN2", target_bir_lowering=False, debug=False)

    la_d = nc.dram_tensor("la", [BPC, J], f32, kind="ExternalInput")
    tip_d = nc.dram_tensor("tipT", [1, 12 * M], f16, kind="ExternalInput")
    out_d = nc.dram_tensor("out", [P, OUTC], f16, kind="ExternalOutput")

    def AP(t, off, dims):
        return bass.AP(t, off, dims)

    SB12 = 12 * S_B

    with TileContext(nc) as tc:
        with tc.tile_pool(name="main", bufs=1) as pool, \
             ExitStack() as _loop_ctx:
            if loop_n > 1:
                _loop_ctx.enter_context(tc.For_i(0, loop_n, 1))

            la_t = [pool.tile([P, S_B * J], f32, name=f"la{s}")
                    for s in range(NSUB)]
            th_t = [pool.tile([P, 3 * M], f16, name=f"th{s}")
                    for s in range(NSUB)]
            ab_t = [pool.tile([P, 3 * M], f16, name=f"ab{s}")
                    for s in range(NSUB)]
            sin_t = [pool.tile([P, 3 * M], f16, name=f"sin{s}")
                     for s in range(NSUB)]
            cos_t = [pool.tile([P, 3 * M], f16, name=f"cos{s}")
                     for s in range(NSUB)]
            tip_t = pool.tile([P, 12 * M], f16, name="tipT")
            loc_t = [pool.tile([P, 12 * M], f16, name=f"locT{s}")
                     for s in range(NSUB)]
            lR_t = [pool.tile([P, 36 * M], f16, name=f"lR{s}")
                    for s in range(NSUB)]
            w_t = [pool.tile([P, 255 * SB12], f16, name=f"w{s}")
                   for s in range(NSUB)]
            t_t = [pool.tile([P, 64 * SB12], f16, name=f"t{s}")
                   for s in range(NSUB)]
            # BC scratch shared across subs (lifetimes serialize)
            tA_t = pool.tile([P, 4 * M], f16, tag="tA", name="tA")
            r0_t = pool.tile([P, 4 * M], f16, tag="r0", name="r0")
            r1_t = pool.tile([P, 4 * M], f16, tag="r1", name="r1")
            q2_t = pool.tile([P, 4 * M], f16, tag="q2", name="q2")
            hpi_t = pool.tile([P, 1], f32, name="hpi")

            nc.gpsimd.memset(hpi_t[:], PI / 2.0)
            # warm the ACT function tables while input DMAs run
            warm_t = pool.tile([P, 1], f32, name="warm")
            nc.scalar.activation(warm_t[:], hpi_t[:], AFT.Tanh)
            nc.scalar.activation(warm_t[:], hpi_t[:], AFT.Sin)

            # ---------------- input DMAs ----------------
            la_v = la_d[:].rearrange("(s l p) j -> p s l j", p=P, l=S_B)
            for s in range(NSUB):
                for ln in range(S_B):
                    nc.sync.dma_start(la_t[s][:, ln * J:(ln + 1) * J],
                                      la_v[:, s, ln])
            tq = (12 * M) // 4
            for c in range(4):
                tip_src = AP(tip_d, c * tq, [[0, P], [1, tq]])
                nc.sync.dma_start(tip_t[:, c * tq:(c + 1) * tq], tip_src)

            act = nc.scalar.activation

            for s in range(NSUB):
                lat = la_t[s][:]
                tht = th_t[s][:]
                # tanh with axis-deinterleave: iterate (a, e, lane)
                th_out = AP(tht.tensor, tht.offset,
                            [list(tht.ap[0]), [M, 3], [S_B, E], [1, S_B]])
                la_in = AP(lat.tensor, lat.offset,
                           [list(lat.ap[0]), [1, 3], [3, E], [J, S_B]])
                act(th_out, la_in, AFT.Tanh)
                # per-axis |theta| (DVE, int16 mask) + sin/cos, z first
                # (BC triples 1-2 need only the z trig)
                for a in (2, 1, 0):
                    sl = slice(a * M, (a + 1) * M)
                    nc.vector.tensor_scalar(
                        ab_t[s][:, sl].bitcast(i16),
                        th_t[s][:, sl].bitcast(i16),
                        0x7FFF, None, Alu.bitwise_and)
                    act(sin_t[s][:, sl], th_t[s][:, sl], AFT.Sin,
                        scale=sc_const)
                    act(cos_t[s][:, sl], ab_t[s][:, sl], AFT.Sin,
                        bias=hpi_t[:], scale=-sc_const)

            # ---------------- BC: locals into locT layout ----------------
            def trig4(tile, axis):
                t = tile[:]
                return AP(t.tensor, t.offset + axis * M,
                          [list(t.ap[0]), [0, 4], [1, M]])

            def row4(tile, k):
                t = tile[:]
                return AP(t.tensor, t.offset + k * 4 * M,
                          [list(t.ap[0]), [M, 4], [1, M]])

            def scr4(tile):
                t = tile[:]
                return AP(t.tensor, t.offset,
                          [list(t.ap[0]), [M, 4], [1, M]])

            def col_slice(ap, lo_e, n_e):
                dims = [list(d) for d in ap.ap]
                dims[-1] = [dims[-1][0], n_e]
                return AP(ap.tensor, ap.offset + lo_e, dims)

            m_dve = M - int(M * BC_POOL_FRAC)

            for s in range(NSUB):
                sx, sy, sz = (trig4(sin_t[s], a) for a in range(3))
                cx, cy, cz = (trig4(cos_t[s], a) for a in range(3))
                T0, T1, T2 = (row4(tip_t, k) for k in range(3))
                L0, L1, L2 = (row4(loc_t[s], k) for k in range(3))
                r0v, r1v, q2v, tAv = (scr4(t) for t in
                                      (r0_t, r1_t, q2_t, tA_t))

                triples = [
                    (cz, T0, sz, T1, r0v, Alu.subtract, True),
                    (sz, T0, cz, T1, r1v, Alu.add, False),
                    (cy, r0v, sy, T2, L0, Alu.add, False),
                    (sy, r0v, cy, T2, q2v, Alu.subtract, False),
                    (cx, r1v, sx, q2v, L1, Alu.subtract, True),
                    (sx, r1v, cx, q2v, L2, Alu.add, False),
                ]
                for (a, b, c, d, dst, op, ta_first) in triples:
                    for eng, lo_e, n_e in ((nc.vector, 0, m_dve),
                                           (nc.gpsimd, m_dve, M - m_dve)):
                        if n_e <= 0:
                            continue
                        tt = eng.tensor_tensor
                        aa, bb, cc, dd, dd_dst, tv = (
                            col_slice(x, lo_e, n_e)
                            for x in (a, b, c, d, dst, tAv))
                        tt(tv, aa, bb, Alu.mult)
                        tt(dd_dst, cc, dd, Alu.mult)
                        if ta_first:
                            tt(dd_dst, tv, dd_dst, op)
                        else:
                            tt(dd_dst, dd_dst, tv, op)

            # ---------------- lR replication ----------------
            rep_engs = []
            acc = 0.0
            for frac, eng in zip(REP_SPLIT, ("act", "pool", "dve")):
                lo_e = int(M * acc)
                acc += frac
                hi_e = M if acc >= 0.999 else int(M * acc)
                if hi_e > lo_e:
                    rep_engs.append((eng, lo_e, hi_e))

            for s in range(NSUB):
                lt = loc_t[s][:]
                rt = lR_t[s][:]
                for k in range(3):
                    for eng, lo_e, hi_e in rep_engs:
                        n_e = hi_e - lo_e
                        dst = AP(rt.tensor, rt.offset + lo_e * 36 + k * 12,
                                 [list(rt.ap[0]), [36, n_e], [3, 4], [1, 3]])
                        src = AP(lt.tensor, lt.offset + k * 4 * M + lo_e,
                                 [list(lt.ap[0]), [1, n_e], [M, 4], [0, 3]])
                        if eng == "act":
                            nc.scalar.copy(dst, src)
                        elif eng == "pool":
                            nc.gpsimd.tensor_copy(dst, src)
                        else:
                            nc.vector.tensor_copy(dst, src)

            # ---------------- tree ----------------
            for s in range(NSUB):
                wt = w_t[s][:]
                rt = lR_t[s][:]
                tst = t_t[s][:]
                wpd, rpd, tpd = (list(x.ap[0]) for x in (wt, rt, tst))

                def wAP(off, dims):
                    return AP(wt.tensor, wt.offset + off, [list(wpd)] + dims)

                def rAP(off, dims):
                    return AP(rt.tensor, rt.offset + off, [list(rpd)] + dims)

                def tAP(off, dims):
                    return AP(tst.tensor, tst.offset + off,
                              [list(tpd)] + dims)

                def nd(n):
                    return (n - 1) * SB12

                # nodes 1,2 <- locals of edges 0,1 ((l,i) from lR's (k,l))
                for n in (1, 2):
                    dst = wAP(nd(n), [[12, S_B], [3, 4], [1, 3]])
                    src = rAP((n - 1) * S_B * 36,
                              [[36, S_B], [3, 4], [12, 3]])
                    nc.vector.tensor_copy(dst, src)

                levels = [(3, 7), (7, 15), (15, 31), (31, 63), (63, 127),
                          (127, 191), (191, 255), (255, 256)]

                def emit_group(eng, glo, ghi, lo):
                    """Full level-compute for child-node range [glo, ghi)."""
                    gm = ghi - glo
                    if gm <= 0:
                        return
                    tt = eng.tensor_tensor
                    for k in range(3):
                        for side in (0, 1):
                            q = (gm + (1 - side)) // 2
                            if q <= 0:
                                continue
                            gplo = (glo + side - 1) // 2
                            in0 = wAP(nd(gplo) + k * 3,
                                      [[12, S_B * q], [0, 4], [1, 3]])
                            e0 = glo + side - 1
                            in1 = rAP(S_B * e0 * 36 + k * 12,
                                      [[72 * S_B, q], [36, S_B], [1, 12]])
                            if k == 0:
                                dst = wAP(nd(glo + side),
                                          [[24 * S_B, q], [12, S_B],
                                           [1, 12]])
                            else:
                                dst = tAP((glo - lo + side) * SB12,
                                          [[24 * S_B, q], [12, S_B],
                                           [1, 12]])
                            tt(dst, in0, in1, Alu.mult)
                        if k > 0:
                            wa = wAP(nd(glo), [[1, SB12 * gm]])
                            ta = tAP((glo - lo) * SB12, [[1, SB12 * gm]])
                            tt(wa, wa, ta, Alu.add)
                    # translation add: w[child].t += w[parent].t
                    for side in (0, 1):
                        q = (gm + (1 - side)) // 2
                        if q <= 0:
                            continue
                        gplo = (glo + side - 1) // 2
                        wtr = wAP(nd(glo + side) + 9,
                                  [[24 * S_B, q], [12, S_B], [1, 3]])
                        ptr = wAP(nd(gplo) + 9, [[12, S_B * q], [1, 3]])
                        tt(wtr, wtr, ptr, Alu.add)

                for (lo, hi) in levels:
                    m = hi - lo
                    if m >= TREE_POOL_MIN_M and TREE_POOL_FRAC > 0:
                        mid = hi - int(m * TREE_POOL_FRAC)
                        mid += (hi - mid) % 2
                    else:
                        mid = hi
                    emit_group(nc.vector, lo, mid, lo)
                    emit_group(nc.gpsimd, mid, hi, lo)

            # ---------------- output DMAs ----------------
            # staged: nodes 1..126 (cols [0, 126*SB12)) are final after level
            # (63,127); tile deps let those DMAs start while the tree tail
            # still runs.  Remainder flushed after the full tree.
            ov = out_d[:]
            w_cols = 255 * SB12
            c1 = 126 * SB12    # nodes 1..126: final after level (63,127)
            c2 = 190 * SB12    # nodes 127..190: final after (127,191)
            for s in range(NSUB):
                for c0, cn in ((0, c1), (c1, c2 - c1), (c2, w_cols - c2)):
                    dst = AP(ov.tensor, ov.offset + s * w_cols + c0,
                             [list(ov.ap[0]), [1, cn]])
                    nc.sync.dma_start(dst, w_t[s][:, c0:c0 + cn])

    nc.compile()
    return nc


# --------------------------------------------------------------------------- #
# cached PJRT runner (axon path) — compile once, execute per call
# --------------------------------------------------------------------------- #
def _get_runner(sc_const, loop_n=1):
    key = ("runner", round(sc_const, 6), loop_n)
    if key in _state:
        return _state[key]

    import jax
    from jax.sharding import Mesh, PartitionSpec, NamedSharding
    from jax.experimental.shard_map import shard_map
    import concourse.mybir as mybir
    from concourse import bass2jax

    nc = _build_nc(sc_const, loop_n)
    bass2jax.install_neuronx_cc_hook()

    part_name = (nc.partition_id_tensor.name
                 if nc.partition_id_tensor is not None else None)
    in_names, out_names, out_avals = [], [], []
    for alloc in nc.m.functions[0].allocations:
        if not isinstance(alloc, mybir.MemoryLocationSet):
            continue
        name = alloc.memorylocations[0].name
        if alloc.kind == "ExternalInput":
            if name != part_name:
                in_names.append(name)
        elif alloc.kind == "ExternalOutput":
            out_names.append(name)
            out_avals.append(jax.core.ShapedArray(
                tuple(alloc.tensor_shape), mybir.dt.np(alloc.dtype)))
    n_params = len(in_names)
    all_in = in_names + out_names
    if part_name is not None:
        all_in = all_in + [part_name]

    def _body(*args):
        operands = list(args)
        if part_name is not None:
            operands.append(bass2jax.partition_id_tensor())
        outs = bass2jax._bass_exec_p.bind(
            *operands,
            out_avals=tuple(out_avals),
            in_names=tuple(all_in),
            out_names=tuple(out_names),
            lowering_input_output_aliases=(),
            sim_require_finite=True,
            sim_require_nnan=True,
            nc=nc,
        )
        return tuple(outs)

    devices = jax.devices()[:NCORE]
    mesh = Mesh(np.asarray(devices), ("core",))
    nin = n_params + len(out_names)
    sharded = jax.jit(
        shard_map(_body, mesh=mesh,
                  in_specs=(PartitionSpec("core"),) * nin,
                  out_specs=(PartitionSpec("core"),) * len(out_names),
                  check_rep=False),
        donate_argnums=tuple(range(n_params, nin)),
        keep_unused=True,
    )
    shard0 = NamedSharding(mesh, PartitionSpec("core"))

    def _make_zeros():
        return jax.jit(
            lambda: jax.numpy.zeros((NCORE * P, OUTC), np.float16),
            out_shardings=shard0)()

    runner = (sharded, in_names, _make_zeros)
    _state[key] = runner
    return runner


def _prep_tip(tip_to_base):
    # tipT[k, l, e'] with e' = S_B*e + lane (lane-duplicated), fp16 flat
    tip_rows = tip_to_base[:, :3, :].astype(np.float16)      # (E, 3, 4)
    tipT = np.repeat(tip_rows.transpose(1, 2, 0), S_B, axis=-1)  # (3,4,M)
    return np.ascontiguousarray(tipT.reshape(1, 12 * M))


def _run_device(log_angles, tip_to_base, sc_const):
    sharded, in_names, make_zeros = _get_runner(sc_const)
    feed = {
        "la": np.ascontiguousarray(log_angles, dtype=np.float32),
        "tipT": np.broadcast_to(_prep_tip(tip_to_base),
                                (NCORE, 12 * M)).copy(),
    }
    args = [feed[name] for name in in_names]
    out = np.asarray(sharded(*args, make_zeros())[0])
    # out: (NCORE*P, OUTC) fp16, device layout -> (B, N, 4, 4) fp32
    v = out.reshape(NCORE, P, NSUB, 255, S_B, 4, 3)  # (c, p, s, n, lane, l, i)
    rot = v.transpose(0, 2, 4, 1, 3, 6, 5)           # (c, s, lane, p, n, i, l)
    res = np.zeros((B, N, 4, 4), np.float32)
    res[:, 1:, :3, :] = rot.reshape(B, 255, 3, 4).astype(np.float32)
    res[:, 0, 0, 0] = 1.0
    res[:, 0, 1, 1] = 1.0
    res[:, 0, 2, 2] = 1.0
    res[:, :, 3, 3] = 1.0
    return res


# --------------------------------------------------------------------------- #
# public entry point
# --------------------------------------------------------------------------- #
def kernel(log_angles, tip_to_base, rot_axes, rot_constraints):
    log_angles = np.asarray(log_angles)
    tip_to_base = np.asarray(tip_to_base)
    rot_axes = np.asarray(rot_axes)
    rot_constraints = np.asarray(rot_constraints)

    expected_shapes = (log_angles.shape == (B, J)
                       and tip_to_base.shape == (E, 4, 4)
                       and rot_axes.shape == (J, 3)
                       and rot_constraints.shape == (J, 2))
    eye_tiled = np.tile(np.eye(3, dtype=np.float32), (E, 1)) \
        if expected_shapes else None
    euler = expected_shapes and np.allclose(rot_axes, eye_tiled, atol=1e-6)
    if not euler:
        return _np_fallback(log_angles, tip_to_base, rot_axes, rot_constraints)

    sc = rot_constraints[:, 0].astype(np.float32)
    of = rot_constraints[:, 1].astype(np.float32)
    const_ok = (np.all(sc == sc[0]) and np.all(of == 0.0)
                and float(sc[0]) > 1e-3
                and abs(float(sc[0])) <= PI + 1e-4)
    if not const_ok:
        return _np_fallback(log_angles, tip_to_base, rot_axes,
                            rot_constraints)

    return _run_device(log_angles, tip_to_base, float(sc[0]))
